# revision 1
# baseline (speedup 1.0000x reference)
"""Trainium2 Bass kernel for nn_ClassAwareLoss (class-aware frame loss).

Contract: kernel(**inputs) takes the FULL unsharded inputs (numpy arrays,
keyed as in setup_inputs()) and returns the FULL output (a float32 scalar).

Strategy (data-parallel over batch, per the sharding hint):
  - Shard `input`/`target` row-wise across 8 NeuronCores (2048 samples each).
  - Replicate the small tensors (frames^T, per-frame class ids, per-frame
    cosine weights) to every core.
  - Each core computes partial sums of
        caloss_c = sum_b sum_f [class(f)==t_b] * cosine_c[t_b] * (1 - d_bf)^2
        reg_c    = sum_b (||x_b|| - 1)^2
    and the host combines: (sum caloss + 6e-4 * sum reg) / B.

Device algorithm (per core, 2048 samples):
  dots are computed in bf16 on the PE (fp32 accumulate in PSUM); the
  normalization 1/||x|| is folded into the ScalarE pass that computes
  S = (1 - g*r)^2 via activation(Square, scale=-g, bias=1).  The
  class mask and per-frame cosine weight fuse into one DVE
  scalar_tensor_tensor op: w = (frame_class == t) * cosine_c[frame_class],
  and a tensor_tensor_reduce accumulates sum(w * S) per partition.
"""

import sys
import types
from contextlib import ExitStack

sys.path.insert(0, "/opt/trn_rl_repo")

import numpy as np
import ml_dtypes

# ---------------------------------------------------------------------------
# antenv.axon_hooks shim: lets run_bass_kernel_spmd(trace=True) capture NTFF
# profiles under axon.  Harmless when BASS_TRACE is not set.
# ---------------------------------------------------------------------------
try:
    import antenv

    if "antenv.axon_hooks" not in sys.modules:
        _mod = types.ModuleType("antenv.axon_hooks")
        _hook = [None]
        _mod.set_axon_ntff_profile_hook = lambda h: _hook.__setitem__(0, h)
        _mod.get_axon_ntff_profile_hook = lambda: _hook[0]
        sys.modules["antenv.axon_hooks"] = _mod
        antenv.axon_hooks = _mod
        try:
            from trn_agent_boot.trn_boot import _ntff_profile_via_ctypes

            _mod.set_axon_ntff_profile_hook(
                _ntff_profile_via_ctypes("/opt/axon/libaxon_pjrt.so")
            )
        except Exception:
            pass
except Exception:
    pass

import concourse.bass as bass
import concourse.tile as tile
import concourse.bass_utils as bass_utils
from concourse import bacc, mybir

# No cloud bucket in this container; keep artifacts local.
bass_utils.upload_artifacts = lambda tmpdir: "local://" + tmpdir

# ---------------------------------------------------------------------------
# Problem constants (from the reference problem definition; input-independent)
# ---------------------------------------------------------------------------
N_CORES = 8
B = 16384
D = 256
NCLS = 100
F_PARAM = 17
BS = B // N_CORES            # 2048 samples per core
NT = BS // 128               # 16 sample-tiles of 128 per core
F_TOTAL = NCLS * (F_PARAM - 1)  # 1600 frame rows

_CLS_SAMPLES = [5000 - 50 * i for i in range(100)]


def _calc_cls_idx(cls_samples, f):
    nc_ = len(cls_samples)
    n_samples = sum(cls_samples)
    ca_frame_num = [int((f - 2) * nc_ * r / n_samples) + 1 for r in cls_samples]
    over_flow = nc_ * (f - 1) - sum(ca_frame_num)
    for i in range(over_flow):
        ca_frame_num[i] += 1
    ca_frame_num.reverse()
    cls_frame_idx = [sum(ca_frame_num[0:k]) for k in range(nc_ + 1)]
    return cls_frame_idx, ca_frame_num


CLS_FRAME_IDX, CA_FRAME_NUM = _calc_cls_idx(_CLS_SAMPLES, F_PARAM)
FRAME_CLASS = np.repeat(np.arange(NCLS), CA_FRAME_NUM)  # [1600], deterministic

BF16 = mybir.dt.bfloat16
F32 = mybir.dt.float32
AF = mybir.ActivationFunctionType
ALU = mybir.AluOpType

_COMPILED = None   # (nc, meta)
LAST_RESULT = None  # BassKernelResults of the most recent run (for test.py)


def _build_program():
    """Build + compile the SPMD Bass program (one program, run on 8 cores)."""
    nc = bacc.Bacc(
        "TRN2", target_bir_lowering=False, debug=False, num_devices=N_CORES
    )

    # Per-core inputs
    x_bf = nc.dram_tensor("x_bf", [BS, D], BF16, kind="ExternalInput").ap()
    t_f32 = nc.dram_tensor("t_f32", [128, NT], F32, kind="ExternalInput").ap()
    framesT = nc.dram_tensor("framesT", [D, F_TOTAL], BF16, kind="ExternalInput").ap()
    iota_in = nc.dram_tensor("iota_mat", [128, 128], BF16, kind="ExternalInput").ap()
    cos_in = nc.dram_tensor("cosine_mat", [128, 128], BF16, kind="ExternalInput").ap()
    ct_in = nc.dram_tensor("ct_mat", [128, F_TOTAL], BF16, kind="ExternalInput").ap()
    out = nc.dram_tensor("out", [128, 2], F32, kind="ExternalOutput").ap()

    with tile.TileContext(nc) as tc:
        with ExitStack() as ctx:
            const_pool = ctx.enter_context(tc.tile_pool(name="const", bufs=1))
            work_pool = ctx.enter_context(tc.tile_pool(name="work", bufs=1))
            s_pool = ctx.enter_context(tc.tile_pool(name="s", bufs=3))
            w_pool = ctx.enter_context(tc.tile_pool(name="w", bufs=3))
            psum_pool = ctx.enter_context(
                tc.tile_pool(name="psum", bufs=2, space="PSUM")
            )
            psum_g = ctx.enter_context(
                tc.tile_pool(name="psumg", bufs=1, space="PSUM")
            )

            # ---- x transposed first: the dots matmuls gate everything ----
            xt0 = work_pool.tile([128, BS], BF16, tag="xt0")
            xt1 = work_pool.tile([128, BS], BF16, tag="xt1")
            nc.sync.dma_start_transpose(xt0[:], x_bf[:, 0:128])
            nc.scalar.dma_start_transpose(xt1[:], x_bf[:, 128:256])

            # ---- x natural layout [128, NT*D] (tile i at cols i*D..) ----
            xn = work_pool.tile([128, NT * D], BF16, tag="xn")
            nc.sync.dma_start(
                xn[:].rearrange("p (i d) -> p i d", i=NT),
                x_bf.rearrange("(i p) d -> p i d", p=128),
            )

            framesT_sb = const_pool.tile([128, 2 * F_TOTAL], BF16, tag="framesT")
            nc.sync.dma_start(framesT_sb[:, 0:F_TOTAL], framesT[0:128, :])
            nc.sync.dma_start(framesT_sb[:, F_TOTAL : 2 * F_TOTAL], framesT[128:256, :])
            iota_sb = const_pool.tile([128, 128], BF16, tag="iota")
            nc.sync.dma_start(iota_sb[:], iota_in[:])
            cos_sb = const_pool.tile([128, 128], BF16, tag="cos")
            nc.sync.dma_start(cos_sb[:], cos_in[:])
            t_sb = const_pool.tile([128, NT], F32, tag="t")
            nc.sync.dma_start(t_sb[:], t_f32[:])
            ct_sb = const_pool.tile([128, F_TOTAL], BF16, tag="ct")
            nc.sync.dma_start(ct_sb[:], ct_in[:])

            neg_one = const_pool.tile([128, 1], F32, tag="negone")
            nc.vector.memset(neg_one[:], -1.0)

            # ---- per-sample squared norms -> [128, NT] ----
            sq = work_pool.tile([128, NT], F32, tag="sq")
            sq_dump = work_pool.tile([128, D], F32, tag="sqd")
            for i in range(NT):
                nc.scalar.activation(
                    sq_dump[:],
                    xn[:, i * D : (i + 1) * D],
                    AF.Square,
                    accum_out=sq[:, i : i + 1],
                )
            # norm, 1/norm, (norm-1)^2
            norm = work_pool.tile([128, NT], F32, tag="norm")
            nc.scalar.activation(norm[:], sq[:], AF.Sqrt)
            g = work_pool.tile([128, NT], F32, tag="g")
            nc.vector.reciprocal(g[:], norm[:])
            regsq = work_pool.tile([128, NT], F32, tag="regsq")
            nc.scalar.activation(
                regsq[:], norm[:], AF.Square, bias=neg_one[:], scale=1.0
            )
            reg_col = work_pool.tile([128, 1], F32, tag="regcol")
            nc.vector.tensor_reduce(
                out=reg_col[:], in_=regsq[:], axis=mybir.AxisListType.X, op=ALU.add
            )

            # ---- main loop over sample tiles ----
            # caloss = sum_c sum_f CT[c,f] * G[c,f],
            # G[c,f] = sum_b cosine_c[t_b] * [t_b == c] * S[b,f]   (PE matmuls)
            g_ps = psum_g.tile([128, F_TOTAL], F32, tag="G")
            HALVES = [(0, 1024), (1024, F_TOTAL)]
            for i in range(NT):
                # ct_col = cosine_c[t_b]; P = ct_col * onehot(t_b)
                ct_dump = w_pool.tile([128, 128], BF16, tag="ctdump")
                ct_col = w_pool.tile([128, 1], F32, tag="ctcol")
                nc.vector.scalar_tensor_tensor(
                    out=ct_dump[:], in0=iota_sb[:], scalar=t_sb[:, i : i + 1],
                    in1=cos_sb[:], op0=ALU.is_equal, op1=ALU.mult,
                    accum_out=ct_col[:],
                )
                p_tile = w_pool.tile([128, 128], BF16, tag="p")
                nc.vector.tensor_scalar(
                    out=p_tile[:], in0=iota_sb[:],
                    scalar1=t_sb[:, i : i + 1], scalar2=ct_col[:],
                    op0=ALU.is_equal, op1=ALU.mult,
                )
                for (flo, fhi) in HALVES:
                    hw_ = fhi - flo
                    dots = psum_pool.tile([128, hw_], F32, tag="dots")
                    for c0 in range(flo, fhi, 512):
                        c1 = min(c0 + 512, fhi)
                        nc.tensor.matmul(
                            dots[:, c0 - flo : c1 - flo],
                            lhsT=xt0[:, i * 128 : (i + 1) * 128],
                            rhs=framesT_sb[:, c0:c1],
                            start=True,
                            stop=False,
                        )
                    for c0 in range(flo, fhi, 512):
                        c1 = min(c0 + 512, fhi)
                        nc.tensor.matmul(
                            dots[:, c0 - flo : c1 - flo],
                            lhsT=xt1[:, i * 128 : (i + 1) * 128],
                            rhs=framesT_sb[:, F_TOTAL + c0 : F_TOTAL + c1],
                            start=False,
                            stop=True,
                        )
                    # S = (g*r - 1)^2  (ScalarE: PSUM -> SBUF bf16)
                    s_tile = s_pool.tile([128, hw_], BF16, tag="s")
                    nc.scalar.activation(
                        s_tile[:], dots[:], AF.Square,
                        bias=neg_one[:], scale=g[:, i : i + 1],
                    )
                    # G[:, chunk] += P^T @ S
                    for c0 in range(flo, fhi, 512):
                        c1 = min(c0 + 512, fhi)
                        nc.tensor.matmul(
                            g_ps[:, c0:c1],
                            lhsT=p_tile[:],
                            rhs=s_tile[:, c0 - flo : c1 - flo],
                            start=(i == 0),
                            stop=(i == NT - 1),
                            skip_group_check=True,
                        )

            # total caloss per class-partition: sum_f CT * G
            g_dump = w_pool.tile([128, F_TOTAL], BF16, tag="gdump")
            cal_col = work_pool.tile([128, 1], F32, tag="calcol")
            nc.vector.scalar_tensor_tensor(
                out=g_dump[:], in0=ct_sb[:], scalar=1.0, in1=g_ps[:],
                op0=ALU.mult, op1=ALU.mult, accum_out=cal_col[:],
            )
            res_sb = work_pool.tile([128, 2], F32, tag="res")
            nc.vector.tensor_copy(res_sb[:, 0:1], cal_col[:])
            nc.vector.tensor_copy(res_sb[:, 1:2], reg_col[:])
            nc.sync.dma_start(out[:], res_sb[:])

    nc.compile()
    return nc


def _prepare_inputs(inputs):
    x = np.asarray(inputs["input"], dtype=np.float32)        # [B, D]
    frames = np.asarray(inputs["frames"], dtype=np.float32)  # [F, D]
    cosine_c = np.asarray(inputs["cosine_c"], dtype=np.float32)  # [NCLS]
    target = np.asarray(inputs["target"])                    # [B] int

    x_bf = x.astype(ml_dtypes.bfloat16)
    framesT = np.ascontiguousarray(frames.T).astype(ml_dtypes.bfloat16)  # [D, F]
    iota_mat = np.ascontiguousarray(
        np.broadcast_to(
            np.arange(128, dtype=np.float32).astype(ml_dtypes.bfloat16), (128, 128)
        )
    )
    cos_pad = np.zeros(128, np.float32)
    cos_pad[:NCLS] = cosine_c
    cosine_mat = np.ascontiguousarray(
        np.broadcast_to(cos_pad.astype(ml_dtypes.bfloat16), (128, 128))
    )
    ct_mat = np.zeros((128, F_TOTAL), np.float32)
    ct_mat[FRAME_CLASS, np.arange(F_TOTAL)] = 1.0
    ct_mat = ct_mat.astype(ml_dtypes.bfloat16)

    in_maps = []
    for c in range(N_CORES):
        sl = slice(c * BS, (c + 1) * BS)
        tc_ = target[sl].astype(np.float32).reshape(NT, 128).T
        # negate target? no: t values compared with fc via is_equal.
        in_maps.append(
            {
                "x_bf": np.ascontiguousarray(x_bf[sl]),
                "t_f32": np.ascontiguousarray(tc_),
                "framesT": framesT,
                "iota_mat": iota_mat,
                "cosine_mat": cosine_mat,
                "ct_mat": ct_mat,
            }
        )
    return in_maps


def kernel(**inputs):
    global _COMPILED, LAST_RESULT
    if _COMPILED is None:
        _COMPILED = _build_program()
    nc = _COMPILED

    in_maps = _prepare_inputs(inputs)
    res = bass_utils.run_bass_kernel_spmd(
        nc, in_maps, core_ids=list(range(N_CORES))
    )
    LAST_RESULT = res

    caloss = 0.0
    reg = 0.0
    for c in range(N_CORES):
        o = res.results[c]["out"].astype(np.float64)
        caloss += o[:, 0].sum()
        reg += o[:, 1].sum()
    val = (caloss + 0.0006 * reg) / B
    return np.float32(val)



# revision 10
# speedup vs baseline: 2.1810x; 2.1810x over previous
"""Trainium2 Bass kernel for nn_ClassAwareLoss (class-aware frame loss).

Contract: kernel(**inputs) takes the FULL unsharded inputs (numpy arrays,
keyed as in setup_inputs()) and returns the FULL output (a float32 scalar).

Strategy (data-parallel over batch, per the sharding hint), v2:
  - Sort samples by target class on the host (index prep only), shard the
    sorted batch row-wise across 8 NeuronCores (2048 samples each).
  - Key algebraic fact: sample b only needs dots with the ~16 frames of its
    own class.  Sorted 128-sample tiles span at most 3 consecutive classes
    (uniform targets), so each tile's matmul uses a gathered frame window of
    3 class-slots x 31 columns = 93 instead of all 1600 frames (~17x less
    PE work; the kernel becomes DMA/memory bound).
  - Window layout: slot k of tile i holds the frames of class c_lo(i)+k,
    zero-padded to 31 columns.  The slot-id pattern (0,0,..,1,1,..,2,2,..)
    is tile-invariant, so the class mask is (slot_id == local_target) with
    one tiny constant matrix.  Zero-pad frames give dots==0 exactly, so each
    pad column in a sample's own slot contributes exactly (0-1)^2 = 1; the
    host sends the per-sample pad count and the kernel subtracts it.

Device algorithm (per core, 2048 samples, 16 tiles of 128):
  sq_b   = sum_d x^2          (split across DVE / GpSimd / Scalar engines)
  g_b    = 1/sqrt(sq), reg_b = (sqrt(sq)-1)^2
  r_bf   = x . frames[window]             (PE, bf16, fp32 PSUM)
  S_bf   = (g_b * r - 1)^2                (ScalarE activation, per-tile)
  cal_b  = sum_f [slot(f)==tl_b] * S_bf   (DVE scalar_tensor_tensor accum)
  caloss = sum_b cvec_b * (cal_b - npad_b);  host sums partials over cores.
"""

import sys
import types
from contextlib import ExitStack

sys.path.insert(0, "/opt/trn_rl_repo")

import numpy as np
import ml_dtypes

# ---------------------------------------------------------------------------
# antenv.axon_hooks shim: lets run_bass_kernel_spmd(trace=True) capture NTFF
# profiles under axon.  Harmless when BASS_TRACE is not set.
# ---------------------------------------------------------------------------
try:
    import antenv

    if "antenv.axon_hooks" not in sys.modules:
        _mod = types.ModuleType("antenv.axon_hooks")
        _hook = [None]
        _mod.set_axon_ntff_profile_hook = lambda h: _hook.__setitem__(0, h)
        _mod.get_axon_ntff_profile_hook = lambda: _hook[0]
        sys.modules["antenv.axon_hooks"] = _mod
        antenv.axon_hooks = _mod
        try:
            from trn_agent_boot.trn_boot import _ntff_profile_via_ctypes

            _mod.set_axon_ntff_profile_hook(
                _ntff_profile_via_ctypes("/opt/axon/libaxon_pjrt.so")
            )
        except Exception:
            pass
except Exception:
    pass

import concourse.bass as bass
import concourse.tile as tile
import concourse.bass_utils as bass_utils
from concourse import bacc, mybir

# No cloud bucket in this container; keep artifacts local.
bass_utils.upload_artifacts = lambda tmpdir: "local://" + tmpdir

# ---------------------------------------------------------------------------
# Problem constants (input-independent; from the reference problem definition)
# ---------------------------------------------------------------------------
N_CORES = 8
B = 16384
D = 256
NCLS = 100
F_PARAM = 17
BS = B // N_CORES            # 2048 samples per core
NT = BS // 128               # 16 sample-tiles of 128 per core
SW = 31                      # slot width = max frames per class
NSLOT = 3                    # max classes spanned by a sorted 128-sample tile
W = NSLOT * SW               # 93 frame columns per tile window

_CLS_SAMPLES = [5000 - 50 * i for i in range(100)]


def _calc_cls_idx(cls_samples, f):
    nc_ = len(cls_samples)
    n_samples = sum(cls_samples)
    ca_frame_num = [int((f - 2) * nc_ * r / n_samples) + 1 for r in cls_samples]
    over_flow = nc_ * (f - 1) - sum(ca_frame_num)
    for i in range(over_flow):
        ca_frame_num[i] += 1
    ca_frame_num.reverse()
    cls_frame_idx = [sum(ca_frame_num[0:k]) for k in range(nc_ + 1)]
    return cls_frame_idx, ca_frame_num


CLS_FRAME_IDX, CA_FRAME_NUM = _calc_cls_idx(_CLS_SAMPLES, F_PARAM)
CA_ARR = np.asarray(CA_FRAME_NUM, dtype=np.int64)       # [100] frames/class
CFI_ARR = np.asarray(CLS_FRAME_IDX, dtype=np.int64)     # [101] prefix index

BF16 = mybir.dt.bfloat16
F32 = mybir.dt.float32
AF = mybir.ActivationFunctionType
ALU = mybir.AluOpType

# norm reduction: which engine handles which sample-tile
# (GpSimd/Pool cannot run TensorScalarPtr -> only DVE and ScalarE do norms)
NORM_DVE = ()
NORM_ACT = tuple(range(16))

_COMPILED = None   # compiled Bacc program
LAST_RESULT = None  # BassKernelResults of the most recent run (for test.py)


def _build_program():
    """Build + compile the SPMD Bass program (one program, run on 8 cores)."""
    nc = bacc.Bacc(
        "TRN2", target_bir_lowering=False, debug=False, num_devices=N_CORES
    )

    # Per-core inputs
    xt0_in = nc.dram_tensor("xt0", [128, BS], BF16, kind="ExternalInput").ap()
    xt1_in = nc.dram_tensor("xt1", [128, BS], BF16, kind="ExternalInput").ap()
    xn_in = nc.dram_tensor("xn", [BS, D], BF16, kind="ExternalInput").ap()
    fw0_in = nc.dram_tensor("fw0", [128, NT * W], BF16, kind="ExternalInput").ap()
    fw1_in = nc.dram_tensor("fw1", [128, NT * W], BF16, kind="ExternalInput").ap()
    fcb_in = nc.dram_tensor("fcb", [128, W], BF16, kind="ExternalInput").ap()
    tl_in = nc.dram_tensor("tl", [128, NT], F32, kind="ExternalInput").ap()
    cvec_in = nc.dram_tensor("cvec", [128, NT], F32, kind="ExternalInput").ap()
    npad_in = nc.dram_tensor("npad", [128, NT], F32, kind="ExternalInput").ap()
    out = nc.dram_tensor("out", [128, 2], F32, kind="ExternalOutput").ap()

    with tile.TileContext(nc) as tc:
        with ExitStack() as ctx:
            const_pool = ctx.enter_context(tc.tile_pool(name="const", bufs=1))
            work_pool = ctx.enter_context(tc.tile_pool(name="work", bufs=1))
            s_pool = ctx.enter_context(tc.tile_pool(name="s", bufs=3))
            d_pool = ctx.enter_context(tc.tile_pool(name="d", bufs=2))
            psum_pool = ctx.enter_context(
                tc.tile_pool(name="psum", bufs=4, space="PSUM")
            )

            # ---- DMAs, spread across the sync / scalar / gpsimd queues ----
            # sync queue: x transposed (gates the matmuls) in quarters
            xt0 = work_pool.tile([128, BS], BF16, tag="xt0")
            xt1 = work_pool.tile([128, BS], BF16, tag="xt1")
            Q = BS // 2
            for q in range(2):
                nc.sync.dma_start(xt0[:, q * Q:(q + 1) * Q], xt0_in[:, q * Q:(q + 1) * Q])
                nc.sync.dma_start(xt1[:, q * Q:(q + 1) * Q], xt1_in[:, q * Q:(q + 1) * Q])

            # frame windows (sync queue; gpsimd SWDGE DMAs hang under axon)
            fw0 = work_pool.tile([128, NT * W], BF16, tag="fw0")
            fw1 = work_pool.tile([128, NT * W], BF16, tag="fw1")
            nc.sync.dma_start(fw0[:], fw0_in[:])
            nc.sync.dma_start(fw1[:], fw1_in[:])

            # scalar queue: x natural (for norms, in halves) + small tensors
            xn = work_pool.tile([128, NT * D], BF16, tag="xn")
            H = NT // 2
            for h in range(2):
                nc.scalar.dma_start(
                    xn[:, h * H * D:(h + 1) * H * D].rearrange(
                        "p (i d) -> p i d", i=H
                    ),
                    xn_in.rearrange("(i p) d -> p i d", p=128)[
                        :, h * H:(h + 1) * H, :
                    ],
                )
            fcb = const_pool.tile([128, W], BF16, tag="fcb")
            nc.scalar.dma_start(fcb[:], fcb_in[:])
            tl = const_pool.tile([128, NT], F32, tag="tl")
            nc.scalar.dma_start(tl[:], tl_in[:])
            cvec = const_pool.tile([128, NT], F32, tag="cvec")
            nc.scalar.dma_start(cvec[:], cvec_in[:])
            npad = const_pool.tile([128, NT], F32, tag="npad")
            nc.scalar.dma_start(npad[:], npad_in[:])

            neg_one = const_pool.tile([128, 1], F32, tag="negone")
            nc.vector.memset(neg_one[:], -1.0)

            # ---- per-sample squared norms sq[:, i], split across engines ----
            sq = work_pool.tile([128, NT], F32, tag="sq")
            for i in NORM_DVE:
                dump = d_pool.tile([128, D], BF16, tag="ndv")
                nc.vector.tensor_tensor_reduce(
                    out=dump[:],
                    in0=xn[:, i * D:(i + 1) * D],
                    in1=xn[:, i * D:(i + 1) * D],
                    scale=1.0,
                    scalar=0.0,
                    op0=ALU.mult,
                    op1=ALU.add,
                    accum_out=sq[:, i:i + 1],
                )
            for i in NORM_ACT:
                dump = d_pool.tile([128, D], F32, tag="nac")
                nc.scalar.activation(
                    dump[:],
                    xn[:, i * D:(i + 1) * D],
                    AF.Square,
                    accum_out=sq[:, i:i + 1],
                )

            # norm, 1/norm, (norm-1)^2 -- in halves so tile 0's S-pass can
            # start before the second half of the norms is done
            norm = work_pool.tile([128, NT], F32, tag="norm")
            g = work_pool.tile([128, NT], F32, tag="g")
            for h in range(2):
                sl = slice(h * H, (h + 1) * H)
                nc.scalar.activation(norm[:, sl], sq[:, sl], AF.Sqrt)
                nc.vector.reciprocal(g[:, sl], norm[:, sl])
            regsq = work_pool.tile([128, NT], F32, tag="regsq")
            nc.scalar.activation(
                regsq[:], norm[:], AF.Square, bias=neg_one[:], scale=1.0
            )
            reg_col = work_pool.tile([128, 1], F32, tag="regcol")
            nc.vector.tensor_reduce(
                out=reg_col[:], in_=regsq[:], axis=mybir.AxisListType.X, op=ALU.add
            )

            # ---- main loop: windowed dots -> S -> masked reduce ----
            cal16 = work_pool.tile([128, NT], F32, tag="cal16")
            for i in range(NT):
                dots = psum_pool.tile([128, W], F32, tag="dots")
                nc.tensor.matmul(
                    dots[:],
                    lhsT=xt0[:, i * 128:(i + 1) * 128],
                    rhs=fw0[:, i * W:(i + 1) * W],
                    start=True,
                    stop=False,
                )
                nc.tensor.matmul(
                    dots[:],
                    lhsT=xt1[:, i * 128:(i + 1) * 128],
                    rhs=fw1[:, i * W:(i + 1) * W],
                    start=False,
                    stop=True,
                )
                s_t = s_pool.tile([128, W], BF16, tag="s")
                nc.scalar.activation(
                    s_t[:], dots[:], AF.Square, bias=neg_one[:], scale=g[:, i:i + 1]
                )
                dump_s = s_pool.tile([128, W], BF16, tag="sd")
                nc.vector.scalar_tensor_tensor(
                    out=dump_s[:],
                    in0=fcb[:],
                    scalar=tl[:, i:i + 1],
                    in1=s_t[:],
                    op0=ALU.is_equal,
                    op1=ALU.mult,
                    accum_out=cal16[:, i:i + 1],
                )

            # ---- finalize: caloss_col = sum_i cvec * (cal16 - npad) ----
            diff = work_pool.tile([128, NT], F32, tag="diff")
            nc.vector.scalar_tensor_tensor(
                out=diff[:],
                in0=cal16[:],
                scalar=1.0,
                in1=npad[:],
                op0=ALU.mult,
                op1=ALU.subtract,
            )
            caldump = work_pool.tile([128, NT], F32, tag="caldump")
            cal_col = work_pool.tile([128, 1], F32, tag="calcol")
            nc.vector.scalar_tensor_tensor(
                out=caldump[:],
                in0=diff[:],
                scalar=1.0,
                in1=cvec[:],
                op0=ALU.mult,
                op1=ALU.mult,
                accum_out=cal_col[:],
            )
            res_sb = work_pool.tile([128, 2], F32, tag="res")
            nc.vector.tensor_copy(res_sb[:, 0:1], cal_col[:])
            nc.vector.tensor_copy(res_sb[:, 1:2], reg_col[:])
            nc.sync.dma_start(out[:], res_sb[:])

    nc.compile()
    return nc


def _prepare_inputs(inputs):
    x = np.asarray(inputs["input"], dtype=np.float32)        # [B, D]
    frames = np.asarray(inputs["frames"], dtype=np.float32)  # [F, D]
    cosine_c = np.asarray(inputs["cosine_c"], dtype=np.float32)  # [NCLS]
    target = np.asarray(inputs["target"]).astype(np.int64)   # [B]

    # sort samples by class (host index prep; the sum is permutation-invariant)
    perm = np.argsort(target, kind="stable")
    t_s = target[perm]
    x_bf = x[perm].astype(ml_dtypes.bfloat16)                # [B, D]
    framesT_bf = np.ascontiguousarray(frames.T).astype(ml_dtypes.bfloat16)

    fcb_row = (np.arange(W) // SW).astype(np.float32)
    fcb = np.ascontiguousarray(
        np.broadcast_to(fcb_row.astype(ml_dtypes.bfloat16), (128, W))
    )

    in_maps = []
    for c in range(N_CORES):
        sl = slice(c * BS, (c + 1) * BS)
        xc = x_bf[sl]                            # [2048, 256] bf16
        tc = t_s[sl]                             # [2048] int64, sorted
        xt = np.ascontiguousarray(xc.T)          # [256, 2048] bf16

        tt = tc.reshape(NT, 128)                 # sample (i, p) = row i*128+p
        c_lo = tt[:, 0]                          # [NT] min class per tile
        span = tt[:, -1] - c_lo
        if int(span.max()) >= NSLOT:
            raise ValueError(
                "target distribution too skewed: a 128-sample tile spans "
                f"{int(span.max()) + 1} classes (> {NSLOT} slots)"
            )
        tl = np.ascontiguousarray((tt - c_lo[:, None]).astype(np.float32).T)
        cvec = np.ascontiguousarray(cosine_c[tt].astype(np.float32).T)
        npad = np.ascontiguousarray((SW - CA_ARR[tt]).astype(np.float32).T)

        fw = np.zeros((2, 128, NT * W), dtype=ml_dtypes.bfloat16)
        for i in range(NT):
            for k in range(NSLOT):
                cls = int(c_lo[i]) + k
                if cls >= NCLS:
                    break
                n = int(CA_ARR[cls])
                f0 = int(CFI_ARR[cls])
                col = i * W + k * SW
                fw[0, :, col:col + n] = framesT_bf[0:128, f0:f0 + n]
                fw[1, :, col:col + n] = framesT_bf[128:256, f0:f0 + n]

        in_maps.append(
            {
                "xt0": xt[0:128],
                "xt1": np.ascontiguousarray(xt[128:256]),
                "xn": xc,
                "fw0": np.ascontiguousarray(fw[0]),
                "fw1": np.ascontiguousarray(fw[1]),
                "fcb": fcb,
                "tl": tl,
                "cvec": cvec,
                "npad": npad,
            }
        )
    return in_maps


def kernel(**inputs):
    global _COMPILED, LAST_RESULT
    if _COMPILED is None:
        _COMPILED = _build_program()
    nc = _COMPILED

    in_maps = _prepare_inputs(inputs)
    res = bass_utils.run_bass_kernel_spmd(
        nc, in_maps, core_ids=list(range(N_CORES))
    )
    LAST_RESULT = res

    caloss = 0.0
    reg = 0.0
    for c in range(N_CORES):
        o = res.results[c]["out"].astype(np.float64)
        caloss += o[:, 0].sum()
        reg += o[:, 1].sum()
    val = (caloss + 0.0006 * reg) / B
    return np.float32(val)


# revision 13
# speedup vs baseline: 2.8606x; 1.3116x over previous
"""Trainium2 Bass kernel for nn_ClassAwareLoss (class-aware frame loss).

Contract: kernel(**inputs) takes the FULL unsharded inputs (numpy arrays,
keyed as in setup_inputs()) and returns the FULL output (a float32 scalar).

Strategy (data-parallel over batch, per the sharding hint):
  - Sort samples by target class on the host (index prep only), shard the
    sorted batch row-wise across 8 NeuronCores (2048 samples each).
  - Sample b only needs dots with the ~16 frames of its own class; sorted
    128-sample tiles span <= 3 consecutive classes, so each tile's matmul
    uses a gathered window of 3 class-slots x 31 cols = 93 frames instead
    of all 1600 (~17x less PE work -> memory-bound kernel).
  - The per-(sample, frame-col) weight w = cosine_c[t_b] * [class(col)==t_b]
    is fully host-known (target-dependent only): ship it as a bf16 matrix
    with zeros on pad/foreign columns.  Then
        caloss_core = sum_{b,f} w * (g_b * r_bf - 1)^2
    is a per-partition grand total: a few wide DVE multiply-accumulate ops,
    no per-tile partials needed.
  - Dots run in fp8 (e4m3) with DoubleRow perf mode: one 256-contraction
    matmul per tile (2x PE throughput, half the x/frames DMA bytes).
    fp8 rounding only perturbs the O(1e-3)-accuracy dots; norms use bf16.

Device algorithm (per core, 2048 samples, 16 tiles of 128):
  x2   = xn * xn                    (DVE, 2 wide STTs, bf16)
  sq   = group-reduce(x2)           (DVE 3D tensor_reduce, [128,16])
  norm = sqrt(sq); g = 1/norm       (ScalarE + DVE reciprocal)
  reg  = sum (norm-1)^2             (ScalarE square + DVE reduce)
  r    = x . frames[window]         (PE fp8 DoubleRow, fp32 PSUM)
  S_i  = (g_i * r - 1)^2            (ScalarE activation per tile)
  cal  = sum w * S                  (DVE: 4 wide STT accumulates)
  out  = [cal_col, reg_col]; host: (sum cal + 6e-4 * sum reg) / B.
"""

import sys
import types
from contextlib import ExitStack

sys.path.insert(0, "/opt/trn_rl_repo")

import numpy as np
import ml_dtypes

# ---------------------------------------------------------------------------
# antenv.axon_hooks shim: lets run_bass_kernel_spmd(trace=True) capture NTFF
# profiles under axon.  Harmless when BASS_TRACE is not set.
# ---------------------------------------------------------------------------
try:
    import antenv

    if "antenv.axon_hooks" not in sys.modules:
        _mod = types.ModuleType("antenv.axon_hooks")
        _hook = [None]
        _mod.set_axon_ntff_profile_hook = lambda h: _hook.__setitem__(0, h)
        _mod.get_axon_ntff_profile_hook = lambda: _hook[0]
        sys.modules["antenv.axon_hooks"] = _mod
        antenv.axon_hooks = _mod
        try:
            from trn_agent_boot.trn_boot import _ntff_profile_via_ctypes

            _mod.set_axon_ntff_profile_hook(
                _ntff_profile_via_ctypes("/opt/axon/libaxon_pjrt.so")
            )
        except Exception:
            pass
except Exception:
    pass

import concourse.bass as bass
import concourse.tile as tile
import concourse.bass_utils as bass_utils
from concourse import bacc, mybir

# No cloud bucket in this container; keep artifacts local.
bass_utils.upload_artifacts = lambda tmpdir: "local://" + tmpdir

# ---------------------------------------------------------------------------
# Problem constants (input-independent; from the reference problem definition)
# ---------------------------------------------------------------------------
N_CORES = 8
B = 16384
D = 256
NCLS = 100
F_PARAM = 17
BS = B // N_CORES            # 2048 samples per core
NT = BS // 128               # 16 sample-tiles of 128 per core
SW = 32                      # slot width (max 31 frames/class + pad; %16 for fp8)
NSLOT = 3                    # max classes spanned by a sorted 128-sample tile
W = NSLOT * SW               # 96 frame columns per tile window

USE_FP8 = True               # fp8 e4m3 + DoubleRow for the dots matmul

_CLS_SAMPLES = [5000 - 50 * i for i in range(100)]


def _calc_cls_idx(cls_samples, f):
    nc_ = len(cls_samples)
    n_samples = sum(cls_samples)
    ca_frame_num = [int((f - 2) * nc_ * r / n_samples) + 1 for r in cls_samples]
    over_flow = nc_ * (f - 1) - sum(ca_frame_num)
    for i in range(over_flow):
        ca_frame_num[i] += 1
    ca_frame_num.reverse()
    cls_frame_idx = [sum(ca_frame_num[0:k]) for k in range(nc_ + 1)]
    return cls_frame_idx, ca_frame_num


CLS_FRAME_IDX, CA_FRAME_NUM = _calc_cls_idx(_CLS_SAMPLES, F_PARAM)
CA_ARR = np.asarray(CA_FRAME_NUM, dtype=np.int64)       # [100] frames/class
CFI_ARR = np.asarray(CLS_FRAME_IDX, dtype=np.int64)     # [101] prefix index

BF16 = mybir.dt.bfloat16
F32 = mybir.dt.float32
FP8 = mybir.dt.float8e4
AF = mybir.ActivationFunctionType
ALU = mybir.AluOpType

NP_FP8 = ml_dtypes.float8_e4m3
NP_BF16 = ml_dtypes.bfloat16

_COMPILED = None   # compiled Bacc program
LAST_RESULT = None  # BassKernelResults of the most recent run (for test.py)


def _build_program():
    """Build + compile the SPMD Bass program (one program, run on 8 cores)."""
    nc = bacc.Bacc(
        "TRN2", target_bir_lowering=False, debug=False, num_devices=N_CORES
    )

    xdt = FP8 if USE_FP8 else BF16
    # Per-core inputs
    if USE_FP8:
        # DoubleRow layouts: per tile i, lhsT cols [i*256, i*256+128) hold
        # contraction chunk 0 (d=0..127), next 128 hold chunk 1; rhs cols
        # [i*2W, i*2W+W) chunk 0, next W chunk 1.
        xt_in = nc.dram_tensor("xt", [128, NT * 256], xdt, kind="ExternalInput").ap()
        fw_in = nc.dram_tensor("fw", [128, NT * 2 * W], xdt, kind="ExternalInput").ap()
    else:
        xt0_in = nc.dram_tensor("xt0", [128, BS], xdt, kind="ExternalInput").ap()
        xt1_in = nc.dram_tensor("xt1", [128, BS], xdt, kind="ExternalInput").ap()
        fw0_in = nc.dram_tensor("fw0", [128, NT * W], xdt, kind="ExternalInput").ap()
        fw1_in = nc.dram_tensor("fw1", [128, NT * W], xdt, kind="ExternalInput").ap()
    xn_in = nc.dram_tensor("xn", [BS, D], BF16, kind="ExternalInput").ap()
    w_in = nc.dram_tensor("w", [128, NT * W], BF16, kind="ExternalInput").ap()
    out = nc.dram_tensor("out", [128, 2], F32, kind="ExternalOutput").ap()

    H = NT // 2

    with tile.TileContext(nc) as tc:
        with ExitStack() as ctx:
            const_pool = ctx.enter_context(tc.tile_pool(name="const", bufs=1))
            work_pool = ctx.enter_context(tc.tile_pool(name="work", bufs=1))
            psum_pool = ctx.enter_context(
                tc.tile_pool(name="psum", bufs=4, space="PSUM")
            )

            # ---- DMAs: all on the sync queue (Scalar runs only ACTs) ----
            # xn halves first: the norms chain is the critical path.
            xn = work_pool.tile([128, NT * D], BF16, tag="xn")
            if USE_FP8:
                xt = work_pool.tile([128, NT * 256], xdt, tag="xt")
                fw = work_pool.tile([128, NT * 2 * W], xdt, tag="fw")
            else:
                xt0 = work_pool.tile([128, BS], xdt, tag="xt0")
                xt1 = work_pool.tile([128, BS], xdt, tag="xt1")
                fw0 = work_pool.tile([128, NT * W], xdt, tag="fw0")
                fw1 = work_pool.tile([128, NT * W], xdt, tag="fw1")
            w_sb = work_pool.tile([128, NT * W], BF16, tag="w")

            for h in range(2):
                nc.sync.dma_start(
                    xn[:, h * H * D:(h + 1) * H * D].rearrange(
                        "p (i d) -> p i d", i=H
                    ),
                    xn_in.rearrange("(i p) d -> p i d", p=128)[
                        :, h * H:(h + 1) * H, :
                    ],
                )
                if USE_FP8:
                    nc.sync.dma_start(
                        xt[:, h * H * 256:(h + 1) * H * 256],
                        xt_in[:, h * H * 256:(h + 1) * H * 256],
                    )
                else:
                    nc.sync.dma_start(
                        xt0[:, h * H * 128:(h + 1) * H * 128],
                        xt0_in[:, h * H * 128:(h + 1) * H * 128],
                    )
                    nc.sync.dma_start(
                        xt1[:, h * H * 128:(h + 1) * H * 128],
                        xt1_in[:, h * H * 128:(h + 1) * H * 128],
                    )
            if USE_FP8:
                nc.scalar.dma_start(fw[:], fw_in[:])
            else:
                nc.scalar.dma_start(fw0[:], fw0_in[:])
                nc.scalar.dma_start(fw1[:], fw1_in[:])
            nc.scalar.dma_start(w_sb[:], w_in[:])

            neg_one = const_pool.tile([128, 1], F32, tag="negone")
            nc.vector.memset(neg_one[:], -1.0)

            # ---- norms: x^2 (DVE wide STT) -> 3D group reduce -> sqrt/recip
            sq = work_pool.tile([128, NT], F32, tag="sq")
            norm = work_pool.tile([128, NT], F32, tag="norm")
            g = work_pool.tile([128, NT], F32, tag="g")
            x2 = work_pool.tile([128, NT * D], BF16, tag="x2")
            for h in range(2):
                sl = slice(h * H * D, (h + 1) * H * D)
                nc.vector.scalar_tensor_tensor(
                    out=x2[:, sl],
                    in0=xn[:, sl],
                    scalar=1.0,
                    in1=xn[:, sl],
                    op0=ALU.mult,
                    op1=ALU.mult,
                )
                nc.vector.tensor_reduce(
                    out=sq[:, h * H:(h + 1) * H],
                    in_=x2[:, sl].rearrange("p (i d) -> p i d", i=H),
                    axis=mybir.AxisListType.X,
                    op=ALU.add,
                )
                hs = slice(h * H, (h + 1) * H)
                nc.scalar.activation(norm[:, hs], sq[:, hs], AF.Sqrt)
                nc.vector.reciprocal(g[:, hs], norm[:, hs])

            regsq = work_pool.tile([128, NT], F32, tag="regsq")
            nc.scalar.activation(
                regsq[:], norm[:], AF.Square, bias=neg_one[:], scale=1.0
            )
            reg_col = work_pool.tile([128, 1], F32, tag="regcol")
            nc.vector.tensor_reduce(
                out=reg_col[:], in_=regsq[:], axis=mybir.AxisListType.X, op=ALU.add
            )

            # ---- main loop: windowed dots -> S = (g*r - 1)^2 ----
            s_big = work_pool.tile([128, NT * W], BF16, tag="sbig")
            for i in range(NT):
                dots = psum_pool.tile([128, W], F32, tag="dots")
                if USE_FP8:
                    nc.tensor.matmul(
                        dots[:],
                        lhsT=xt[:, i * 256:(i + 1) * 256].rearrange(
                            "p (two m) -> p two m", two=2
                        ),
                        rhs=fw[:, i * 2 * W:(i + 1) * 2 * W].rearrange(
                            "p (two f) -> p two f", two=2
                        ),
                        start=True,
                        stop=True,
                        perf_mode=mybir.MatmulPerfMode.DoubleRow,
                    )
                else:
                    nc.tensor.matmul(
                        dots[:],
                        lhsT=xt0[:, i * 128:(i + 1) * 128],
                        rhs=fw0[:, i * W:(i + 1) * W],
                        start=True,
                        stop=False,
                    )
                    nc.tensor.matmul(
                        dots[:],
                        lhsT=xt1[:, i * 128:(i + 1) * 128],
                        rhs=fw1[:, i * W:(i + 1) * W],
                        start=False,
                        stop=True,
                    )
                nc.scalar.activation(
                    s_big[:, i * W:(i + 1) * W],
                    dots[:],
                    AF.Square,
                    bias=neg_one[:],
                    scale=g[:, i:i + 1],
                )

            # ---- caloss: per-partition grand total of w * S (4 wide STTs)
            NCHUNK = 4
            CW = NT * W // NCHUNK
            calpart = work_pool.tile([128, NCHUNK], F32, tag="calpart")
            sdump = work_pool.tile([128, CW], BF16, tag="sdump")
            for j in range(NCHUNK):
                sl = slice(j * CW, (j + 1) * CW)
                nc.vector.scalar_tensor_tensor(
                    out=sdump[:],
                    in0=s_big[:, sl],
                    scalar=1.0,
                    in1=w_sb[:, sl],
                    op0=ALU.mult,
                    op1=ALU.mult,
                    accum_out=calpart[:, j:j + 1],
                )
            cal_col = work_pool.tile([128, 1], F32, tag="calcol")
            nc.vector.tensor_reduce(
                out=cal_col[:], in_=calpart[:], axis=mybir.AxisListType.X, op=ALU.add
            )

            res_sb = work_pool.tile([128, 2], F32, tag="res")
            nc.vector.tensor_copy(res_sb[:, 0:1], cal_col[:])
            nc.vector.tensor_copy(res_sb[:, 1:2], reg_col[:])
            nc.sync.dma_start(out[:], res_sb[:])

    nc.compile()
    return nc


def _prepare_inputs(inputs):
    x = np.asarray(inputs["input"], dtype=np.float32)        # [B, D]
    frames = np.asarray(inputs["frames"], dtype=np.float32)  # [F, D]
    cosine_c = np.asarray(inputs["cosine_c"], dtype=np.float32)  # [NCLS]
    target = np.asarray(inputs["target"]).astype(np.int64)   # [B]

    # sort samples by class (host index prep; the sum is permutation-invariant)
    perm = np.argsort(target, kind="stable")
    t_s = target[perm]
    x_s = x[perm]
    np_xdt = NP_FP8 if USE_FP8 else NP_BF16
    framesT = np.ascontiguousarray(frames.T)                 # [D, F] f32

    in_maps = []
    for c in range(N_CORES):
        sl = slice(c * BS, (c + 1) * BS)
        xc = x_s[sl]                             # [2048, 256] f32
        tc = t_s[sl]                             # [2048] int64, sorted

        tt = tc.reshape(NT, 128)                 # sample (i, p) = row i*128+p
        c_lo = tt[:, 0]                          # [NT] min class per tile
        span = tt[:, -1] - c_lo
        if int(span.max()) >= NSLOT:
            raise ValueError(
                "target distribution too skewed: a 128-sample tile spans "
                f"{int(span.max()) + 1} classes (> {NSLOT} slots)"
            )

        # frame windows + host-known weight mask (cosine on own-class cols)
        fw = np.zeros((2, 128, NT * W), dtype=np.float32)
        wmat = np.zeros((128, NT * W), dtype=np.float32)
        for i in range(NT):
            for k in range(NSLOT):
                cls = int(c_lo[i]) + k
                if cls >= NCLS:
                    break
                n = int(CA_ARR[cls])
                f0 = int(CFI_ARR[cls])
                col = i * W + k * SW
                fw[0, :, col:col + n] = framesT[0:128, f0:f0 + n]
                fw[1, :, col:col + n] = framesT[128:256, f0:f0 + n]
                rows = tt[i] == cls
                wmat[rows, col:col + n] = cosine_c[cls]

        m = {
            "xn": xc.astype(NP_BF16),
            "w": wmat.astype(NP_BF16),
        }
        xt = np.ascontiguousarray(xc.T).astype(np_xdt)       # [256, 2048]
        if USE_FP8:
            # DoubleRow: [128, NT*256], tile i = [chunk0 128 | chunk1 128]
            xdr = xt.reshape(2, 128, NT, 128).transpose(1, 2, 0, 3)
            m["xt"] = np.ascontiguousarray(xdr.reshape(128, NT * 256))
            fdr = (
                fw.astype(np_xdt)
                .reshape(2, 128, NT, W)
                .transpose(1, 2, 0, 3)
            )
            m["fw"] = np.ascontiguousarray(fdr.reshape(128, NT * 2 * W))
        else:
            m["xt0"] = np.ascontiguousarray(xt[0:128])
            m["xt1"] = np.ascontiguousarray(xt[128:256])
            m["fw0"] = np.ascontiguousarray(fw[0].astype(np_xdt))
            m["fw1"] = np.ascontiguousarray(fw[1].astype(np_xdt))
        in_maps.append(m)
    return in_maps


def kernel(**inputs):
    global _COMPILED, LAST_RESULT
    if _COMPILED is None:
        _COMPILED = _build_program()
    nc = _COMPILED

    in_maps = _prepare_inputs(inputs)
    res = bass_utils.run_bass_kernel_spmd(
        nc, in_maps, core_ids=list(range(N_CORES))
    )
    LAST_RESULT = res

    caloss = 0.0
    reg = 0.0
    for c in range(N_CORES):
        o = res.results[c]["out"].astype(np.float64)
        caloss += o[:, 0].sum()
        reg += o[:, 1].sum()
    val = (caloss + 0.0006 * reg) / B
    return np.float32(val)
